# revision 37
# baseline (speedup 1.0000x reference)
"""GNN message-passing (graph convolution) kernel for 8 Trainium2 NeuronCores.

    out = relu(segment_sum(h[col], row) + bias),  h = x @ W

Strategy (v5, "aggregate-x-then-matmul" + chained dedup): by linearity,
segment_sum(x@W [col], row) = segment_sum(x[col], row) @ W, so the dense
projection is applied AFTER aggregation and the per-edge work is pure data
movement:

  * Host LPT-balances the 20000 nodes into 157 dst blocks of 128 (by degree);
    blocks are assigned contiguously to cores (20/core), so each core
    produces a disjoint output slice - no collectives.
  * Each core's blocks are greedily ordered into a chain maximizing
    consecutive source overlap, and the core consumes ONE gather stream of
    CHAINED per-block segments
    [seg_0 | seg_1 | ... | seg_19]: a segment holds the distinct source
    columns of its block not already emitted in the previous segment, with
    columns also used by the NEXT block placed in a shared tail.  Block q's
    one-hot matmul range covers seg q plus the shared tail of seg q-1, so
    every shared column is gathered once but aggregated into both blocks
    (~20% fewer gathered rows than raw edges).  Only columns with a single
    destination in the current block are shared (keeps its pass schedule
    flat); sources with
    k>=2 destinations inside a block run k one-hot passes and are sorted to
    the front of their segment (per-chunk pass counts are global maxima, so
    the SPMD instruction stream is identical on every core).
  * The gather stream is FLAT: SWDGE dma_gather instructions of 7 chunks
    (896 rows, the ring limit) are issued back to back into an 84-chunk val
    ring, crossing segment boundaries, minimizing the count of gather
    instructions (994ns fixed cost each on the GpSimd engine).  x rows
    (512B fp16) are gathered straight from DRAM.
  * DVE builds one-hot tiles S[e,n] = (iota == rowloc[pass]) in fp16; the PE
    accumulates aggT[f,n] += val[:,c,f]^T @ S over the block's range in PSUM
    fp32 - an exact transposed segment-sum (no PE transposes).
  * DVE copies aggT to SBUF fp16; PE computes out_b = aggT^T @ W + bias
    (bias via an identity-matmul against a broadcast bias tile); ACT applies
    ReLU (fp16 out); DMA out; host re-permutes and upcasts.

Numerics: fp16 operands with fp32 accumulation; one-hot matmuls are exact, so
the only error is fp16 rounding of x, W and the aggregate (~1e-3 relative).
"""

import sys

import numpy as np

sys.path.insert(0, "/opt/trn_rl_repo")

import concourse.bacc as bacc  # noqa: E402
import concourse.bass as bass  # noqa: E402  (engine types)
import concourse.mybir as mybir  # noqa: E402
from concourse.bass_utils import run_bass_kernel_spmd  # noqa: E402

N_NODES = 20000
FIN = 256
FOUT = 128
N_EDGES = 640000

NBLK = 157               # dst blocks of 128 nodes (157*128 = 20096 slots)
NCORES = 8
NB = 20                  # block slots per core (core 7: 17 real + 3 dummy)
NPAIR = NB // 2

S_BUFS = 8               # one-hot tile ring
GRP = 7                  # chunks per gather (896 rows <= 1024-desc SWDGE ring)
VC = 84                  # val ring chunks (multiple of GRP)
NSEM = VC // GRP         # rotating gather-completion semaphores

FP16 = mybir.dt.float16
FP32 = mybir.dt.float32
I16 = mybir.dt.int16


def _host_prep(x, edge_index, weight, bias):
    """Cast operands, balance nodes into blocks, build chained gather streams."""
    import heapq

    x16 = np.ascontiguousarray(np.asarray(x, np.float32).astype(np.float16))
    weight = np.asarray(weight, np.float32)
    bias = np.asarray(bias, np.float32)

    w_t = np.ascontiguousarray(weight.astype(np.float16).reshape(2, 128, 128))
    bias_bc = np.ascontiguousarray(
        np.broadcast_to(bias.astype(np.float16), (128, 128))
    )
    ident = np.eye(128, dtype=np.float16)
    iota16 = np.ascontiguousarray(
        np.broadcast_to(np.arange(128, dtype=np.float16), (128, 128))
    )

    row = np.asarray(edge_index[0]).astype(np.int64)
    col = np.asarray(edge_index[1]).astype(np.int64)

    # LPT-balance nodes into NBLK blocks of <=128 nodes (by degree) to
    # minimize the max edges-per-block.
    deg = np.bincount(row, minlength=N_NODES)
    order = np.argsort(-deg, kind="stable")
    blk_of = np.empty(N_NODES, np.int32)
    slot_of = np.empty(N_NODES, np.int32)
    heap = [(0, b) for b in range(NBLK)]
    heapq.heapify(heap)
    nslots = np.zeros(NBLK, np.int32)
    for n in order:
        load, b = heapq.heappop(heap)
        blk_of[n] = b
        slot_of[n] = nslots[b]
        nslots[b] += 1
        if nslots[b] < 128:
            heapq.heappush(heap, (load + int(deg[n]), b))

    b_of_edge = blk_of[row]
    eorder = np.argsort(b_of_edge, kind="stable")
    cs = col[eorder].astype(np.int32)
    rloc = slot_of[row[eorder]].astype(np.int32)
    counts = np.bincount(b_of_edge, minlength=NBLK)
    starts = np.concatenate([[0], np.cumsum(counts)])

    # Build one dedup'd CHAIN stream per core: segments seg_0..seg_19, one
    # per block slot.  A column of block q already emitted in seg q-1 (and
    # thus covered by range q) is not re-emitted; emitted columns also in
    # block q+1 form the segment's "shared" tail, consumed by range q+1 too.
    def blk_edges(g):
        if g < 0 or g >= NBLK:
            return (np.empty(0, np.int32), np.empty(0, np.int32))
        e0, e1 = int(starts[g]), int(starts[g + 1])
        return cs[e0:e1], rloc[e0:e1]

    # Pre-group every block's edges by column.
    ublk = []
    for g in range(NBLK):
        c_b, r_b = blk_edges(g)
        o2 = np.argsort(c_b, kind="stable")
        sc, sr = c_b[o2], r_b[o2]
        new = np.r_[True, sc[1:] != sc[:-1]] if sc.size else np.empty(0, bool)
        first = np.flatnonzero(new)
        gid = np.cumsum(new) - 1 if sc.size else np.empty(0, np.int64)
        dcnt = np.diff(np.r_[first, sc.size]) if sc.size else np.empty(0, np.int64)
        rank = (np.arange(sc.size) - first[gid]) if sc.size else np.empty(0, np.int64)
        ublk.append((sc[first] if sc.size else np.empty(0, np.int32),
                     dcnt, gid, rank, sr))

    # Order each core's blocks into a chain that maximizes consecutive
    # source overlap (greedy path extension on |U_a cap U_b|).
    chain = np.full((NCORES, NB), -1, np.int64)
    for c in range(NCORES):
        ids = list(range(c * NB, min((c + 1) * NB, NBLK)))
        n = len(ids)
        ov = np.zeros((n, n), np.int64)
        for a in range(n):
            for b in range(a + 1, n):
                w = np.intersect1d(
                    ublk[ids[a]][0], ublk[ids[b]][0], assume_unique=True
                ).size
                ov[a, b] = ov[b, a] = w
        a0, b0 = divmod(int(np.argmax(ov)), n)
        path = [a0, b0]
        used = {a0, b0}
        while len(path) < n:
            rest = [k for k in range(n) if k not in used]
            headw = max(rest, key=lambda k: ov[path[0], k])
            tailw = max(rest, key=lambda k: ov[path[-1], k])
            if ov[path[0], headw] > ov[path[-1], tailw]:
                path.insert(0, headw)
                used.add(headw)
            else:
                path.append(tailw)
                used.add(tailw)
        chain[c, :n] = [ids[k] for k in path]
    # relabel blocks to chain position so the output layout follows
    newpos = np.full(NBLK, -1, np.int64)
    for c in range(NCORES):
        for q in range(NB):
            if chain[c, q] >= 0:
                newpos[chain[c, q]] = c * NB + q

    E = np.empty(0, np.int64)
    seg_n = np.zeros((NCORES, NB), np.int64)    # rows per segment
    seg_only = np.zeros((NCORES, NB), np.int64)  # rows before shared tail
    seg_cols = {}                                # (c,q) -> col values in order
    entries = {}   # (c,q) -> (seg_of_row, u=row//128, part, rank, rl)
    for c in range(NCORES):
        prev_cols = np.empty(0, np.int32)        # emitted shared cols of q-1
        prev_pos = np.empty(0, np.int64)         # their row pos in seg q-1
        for q in range(NB):
            g = int(chain[c, q])
            if g < 0:
                seg_cols[(c, q)] = np.empty(0, np.int32)
                entries[(c, q)] = (E, E, E, E, E)
                prev_cols, prev_pos = np.empty(0, np.int32), E
                continue
            ucols, dcnt, gid, rank, sr = ublk[g]
            ng = ucols.size
            covered = np.isin(ucols, prev_cols)
            emit_idx = np.flatnonzero(~covered)
            emit_cols = ucols[emit_idx]
            gn = int(chain[c, q + 1]) if q + 1 < NB else -1
            if gn >= 0:
                nxt_ucols, nxt_dcnt = ublk[gn][0], ublk[gn][1]
                ip = np.searchsorted(nxt_ucols, emit_cols)
                ip = np.minimum(ip, max(nxt_ucols.size - 1, 0))
                in_next = (nxt_ucols.size > 0) & (nxt_ucols[ip] == emit_cols)
                d_next = np.where(in_next, nxt_dcnt[ip], 0)
            else:
                in_next = np.zeros(emit_cols.size, bool)
                d_next = np.zeros(emit_cols.size, np.int64)
            d_cur = dcnt[emit_idx]
            # share only columns with a single dst in the CURRENT block: the
            # shared tail stays flat (1 pass) for range q; multi-dst-in-next
            # columns only inflate the head of range q+1 (sorted desc there)
            in_next = in_next & (d_cur == 1)
            key = np.where(in_next, np.maximum(d_cur, d_next), d_cur)
            uo = np.lexsort((-key, in_next.astype(np.int8)))
            pos_of_emit = np.empty(emit_cols.size, np.int64)
            pos_of_emit[uo] = np.arange(emit_cols.size)
            seg_cols[(c, q)] = emit_cols[uo]
            seg_n[c, q] = emit_cols.size
            seg_only[c, q] = int((~in_next).sum())
            # per-group row position: covered -> prev seg, else this seg
            grow = np.empty(ng, np.int64)
            gseg = np.empty(ng, np.int64)
            if prev_cols.size:
                pi = np.searchsorted(prev_cols, ucols[covered])
                grow[covered] = prev_pos[pi]
                gseg[covered] = q - 1
            grow[~covered] = pos_of_emit
            gseg[~covered] = q
            entries[(c, q)] = (
                gseg[gid], grow[gid] // 128, grow[gid] % 128,
                rank, sr.astype(np.int64),
            )
            shared = np.flatnonzero(in_next[uo])
            prev_cols = emit_cols[uo][shared]
            po = np.argsort(prev_cols, kind="stable")
            prev_cols = prev_cols[po]
            prev_pos = shared[po]

    segc = np.maximum(seg_n.max(axis=0) + 127, 128) // 128  # chunks per seg
    posq = np.concatenate([[0], np.cumsum(segc)])
    tc = int(posq[-1])                                       # chunks per core
    rs, re = [0] * NB, [0] * NB
    for q in range(NB):
        re[q] = int(posq[q] + segc[q])
        if q == 0:
            rs[q] = 0
        else:
            live = seg_n[:, q - 1] > 0
            off = int((seg_only[live, q - 1] // 128).min()) if live.any() else \
                int(segc[q - 1])
            rs[q] = int(posq[q - 1]) + off

    # Static pass schedules: per-range per-chunk max of dst-counts.
    passes = []
    for q in range(NB):
        pq = np.zeros(re[q] - rs[q], np.int64)
        for c in range(NCORES):
            gseg, u, part, rank, rl = entries[(c, q)]
            if len(gseg) == 0:
                continue
            i = posq[gseg] + u - rs[q]
            np.maximum.at(pq, i, rank + 1)
        passes.append(np.maximum(pq, 1))
    pcums = [np.concatenate([[0], np.cumsum(p)]) for p in passes]
    sbase = np.concatenate([[0], np.cumsum([int(p[-1]) for p in pcums])])
    tslots = int(sbase[-1])

    nidx = tc * 128
    idxc = nidx // 16
    col16 = np.zeros((NCORES, 128, idxc), np.int16)
    rloc16 = np.full((NCORES, 128, tslots), -1.0, np.float32)
    for c in range(NCORES):
        lin_col = np.zeros(nidx, np.int32)
        lin_rl = np.full((tslots, 128), -1.0, np.float32)
        for q in range(NB):
            u0 = seg_cols[(c, q)]
            lin_col[posq[q] * 128:posq[q] * 128 + u0.size] = u0
            gseg, u, part, rank, rl = entries[(c, q)]
            if len(gseg) == 0:
                continue
            i = posq[gseg] + u - rs[q]
            lin_rl[sbase[q] + pcums[q][i] + rank, part] = rl
        # the SWDGE Q7 cores read the indices from different 16-partition
        # groups - replicate the 16-row wrap to all 128
        col16[c] = np.tile(lin_col.reshape(idxc, 16).T.astype(np.int16), (8, 1))
        rloc16[c] = lin_rl.T
    # out_concat[blk*128 + slot] -> node (block ids are already slot order)
    pos = (newpos[blk_of] * 128 + slot_of).astype(np.int64)
    meta = (tc, rs, re, [list(map(int, p)) for p in passes])
    return x16, w_t, bias_bc, ident, iota16, col16, rloc16, meta, pos


def _build_program(meta):
    tc, rs, re, passes = meta
    pcums = []
    for p in passes:
        c = [0]
        for v in p:
            c.append(c[-1] + v)
        pcums.append(c)
    sbase = [0]
    for c in pcums:
        sbase.append(sbase[-1] + c[-1])
    tslots = sbase[-1]
    idxc = tc * 8
    nch = tc                         # global chunk count
    ngat = (nch + GRP - 1) // GRP    # flat gather instructions

    # range of block-slot q: list of (chunk j, passes, rl slot base)
    def q_range(q):
        return [
            (rs[q] + i, passes[q][i], sbase[q] + pcums[q][i])
            for i in range(re[q] - rs[q])
        ]

    def smm_after(q):  # s_smm value after block-slot q's range completes
        return sbase[q + 1]

    # s_smm value at which chunk j is fully consumed (last covering range)
    tgt = [0] * nch
    for q in range(NB):
        for i in range(re[q] - rs[q]):
            tgt[rs[q] + i] = sbase[q] + pcums[q][i + 1]

    def consume_tgt(j):
        return tgt[j]

    # ramp split points: chunks/slots of the first two ranges
    ch0 = re[1]
    sl0 = sbase[2]

    nc = bacc.Bacc("TRN2")

    x_d = nc.dram_tensor("x16", [N_NODES, FIN], FP16, kind="ExternalInput")
    w_d = nc.dram_tensor("w", [2, 128, 128], FP16, kind="ExternalInput")
    bb_d = nc.dram_tensor("bb", [128, 128], FP16, kind="ExternalInput")
    id_d = nc.dram_tensor("ident", [128, 128], FP16, kind="ExternalInput")
    io_d = nc.dram_tensor("iota", [128, 128], FP16, kind="ExternalInput")
    col_d = nc.dram_tensor("col", [128, idxc], I16, kind="ExternalInput")
    rl_d = nc.dram_tensor("rl", [128, tslots], FP32, kind="ExternalInput")
    o_d = nc.dram_tensor("out", [NB * 128, 128], FP16, kind="ExternalOutput")

    from contextlib import ExitStack

    with ExitStack() as es:
        # aggT accumulators: [parity][feature-half], one bank each
        pa = [
            [es.enter_context(nc.psum_tensor(f"pa{k}{h}", [128, 512], FP32))
             for h in range(2)]
            for k in range(2)
        ]
        po = [es.enter_context(nc.psum_tensor(f"po{k}", [128, 512], FP32))
              for k in range(2)]
        w_sb = es.enter_context(nc.sbuf_tensor("w_sb", [128, 2, 128], FP16))
        bb_sb = es.enter_context(nc.sbuf_tensor("bb_sb", [128, 128], FP16))
        id_sb = es.enter_context(nc.sbuf_tensor("id_sb", [128, 128], FP16))
        iota_sb = es.enter_context(nc.sbuf_tensor("iota_sb", [128, 128], FP16))
        col_sb = es.enter_context(
            nc.sbuf_tensor("col_sb", [128, idxc], I16)
        )
        rl_sb = es.enter_context(
            nc.sbuf_tensor("rl_sb", [128, tslots], FP32)
        )
        val_sb = es.enter_context(nc.sbuf_tensor("val_sb", [128, VC, FIN], FP16))
        s_sb = es.enter_context(nc.sbuf_tensor("s_sb", [128, S_BUFS, 128], FP16))
        at_sb = es.enter_context(nc.sbuf_tensor("at_sb", [128, 2, 2, 128], FP16))
        o_sb = es.enter_context(nc.sbuf_tensor("o_sb", [128, 2, 128], FP16))

        s_ld = [es.enter_context(nc.semaphore(f"s_ld{k}")) for k in range(10)]
        s_gat = [
            es.enter_context(nc.semaphore(f"s_gat{k}")) for k in range(NSEM)
        ]
        s_ow = [es.enter_context(nc.semaphore(f"s_ow{k}")) for k in range(2)]
        s_s = es.enter_context(nc.semaphore("s_s"))      # DVE one-hot count
        s_smm = es.enter_context(nc.semaphore("s_smm"))  # PE pass-mm count
        s_vcp = es.enter_context(nc.semaphore("s_vcp"))  # DVE aggT copies
        s_omm = es.enter_context(nc.semaphore("s_omm"))  # PE final-mm count
        s_ocp = es.enter_context(nc.semaphore("s_ocp"))  # ACT relu count
        block = es.enter_context(nc.Block())

        (LD_COL00, LD_COL0, LD_COL1, LD_IO, LD_RL0, LD_RL1, LD_W, LD_W1,
         LD_BB, LD_ID) = range(10)

        @block.sync
        def _(sync):
            # Ramp-critical loads first: pair-0 idx slice gates the first
            # gather; iota + pair-0 rloc gate the first one-hot builds.
            sync.dma_start(
                col_sb[:, 0:ch0 * 8], col_d[:, 0:ch0 * 8]
            ).then_inc(s_ld[LD_COL0], 16)
            sync.dma_start(iota_sb[:, :], io_d[:, :]).then_inc(s_ld[LD_IO], 16)
            sync.dma_start(
                rl_sb[:, 0:sl0], rl_d[:, 0:sl0]
            ).then_inc(s_ld[LD_RL0], 16)
            sync.dma_start(
                col_sb[:, ch0 * 8:], col_d[:, ch0 * 8:]
            ).then_inc(s_ld[LD_COL1], 16)
            sync.dma_start(
                rl_sb[:, sl0:], rl_d[:, sl0:]
            ).then_inc(s_ld[LD_RL1], 16)
            sync.dma_start(w_sb[:, 0, :], w_d[0]).then_inc(s_ld[LD_W], 16)
            sync.dma_start(w_sb[:, 1, :], w_d[1]).then_inc(s_ld[LD_W1], 16)
            sync.dma_start(bb_sb[:, :], bb_d[:, :]).then_inc(s_ld[LD_BB], 16)
            sync.dma_start(id_sb[:, :], id_d[:, :]).then_inc(s_ld[LD_ID], 16)
            for b in range(NB):
                sync.wait_ge(s_ocp, b + 1)
                sync.dma_start(
                    o_d[b * 128:(b + 1) * 128, :], o_sb[:, b % 2, :]
                ).then_inc(s_ow[b % 2], 16)

        @block.gpsimd
        def _(gpsimd):
            gpsimd.wait_ge(s_ld[LD_COL0], 16)
            g_cross = next(g for g in range(ngat + 1) if GRP * g + GRP > ch0)
            for g in range(ngat):
                if g == g_cross:
                    gpsimd.wait_ge(s_ld[LD_COL1], 16)
                j0, j1 = GRP * g, min(GRP * g + GRP, nch)
                if j1 - VC > 0:
                    # val ring slots [j0%VC, ...) held chunks [j0-VC, j1-VC)
                    gpsimd.wait_ge(s_smm, consume_tgt(j1 - VC - 1))
                r = j0 % VC
                gpsimd.dma_gather(
                    val_sb[:, r:r + (j1 - j0), :],
                    x_d[:, :],
                    col_sb[:, j0 * 8:j1 * 8],
                    (j1 - j0) * 128,
                    (j1 - j0) * 128,
                    FIN,
                ).then_inc(s_gat[g % NSEM], 16)

        def pe_final(tensor, q):
            if q == 0:
                for k in (LD_W, LD_W1, LD_BB, LD_ID):
                    tensor.wait_ge(s_ld[k], 16)
            tensor.wait_ge(s_vcp, 2 * (q + 1))
            if q >= 2:
                tensor.wait_ge(s_ocp, q - 1)
            tensor.matmul(
                po[q % 2][:, 0:128], id_sb[:, :], bb_sb[:, :],
                start=True, stop=False,
            )
            tensor.matmul(
                po[q % 2][:, 0:128], at_sb[:, q % 2, 0, :], w_sb[:, 0, :],
                start=False, stop=False,
            )
            tensor.matmul(
                po[q % 2][:, 0:128], at_sb[:, q % 2, 1, :], w_sb[:, 1, :],
                start=False, stop=True,
            ).then_inc(s_omm, 1)

        @block.tensor
        def _(tensor):
            kk = 0
            waited_g = 0
            for q in range(NB):
                rng = q_range(q)
                for i, (j, np_, rlb) in enumerate(rng):
                    g = j // GRP
                    while waited_g <= g:
                        tensor.wait_ge(
                            s_gat[waited_g % NSEM],
                            16 * (waited_g // NSEM + 1),
                        )
                        waited_g += 1
                    if i == 0 and q >= 2:
                        # pa[q%2] fully copied out (block-slot q-2)
                        tensor.wait_ge(s_vcp, 2 * (q - 1))
                    for p in range(np_):
                        tensor.wait_ge(s_s, kk + 1)
                        st = i == 0 and p == 0
                        sp = i == len(rng) - 1 and p == np_ - 1
                        tensor.matmul(
                            pa[q % 2][0][:, 0:128],
                            val_sb[:, j % VC, 0:128],
                            s_sb[:, kk % S_BUFS, :],
                            start=st,
                            stop=sp,
                        )
                        tensor.matmul(
                            pa[q % 2][1][:, 0:128],
                            val_sb[:, j % VC, 128:256],
                            s_sb[:, kk % S_BUFS, :],
                            start=st,
                            stop=sp,
                        ).then_inc(s_smm, 1)
                        kk += 1
                if q >= 1:
                    pe_final(tensor, q - 1)
            pe_final(tensor, NB - 1)

        def dve_copies(vector, q):
            vector.wait_ge(s_smm, smm_after(q))
            if q >= 2:
                # at_sb[q%2] consumed by pe_final(q-2)
                vector.wait_ge(s_omm, q - 1)
            vector.tensor_copy(
                at_sb[:, q % 2, 0, :], pa[q % 2][0][:, 0:128]
            ).then_inc(s_vcp, 1)
            vector.tensor_copy(
                at_sb[:, q % 2, 1, :], pa[q % 2][1][:, 0:128]
            ).then_inc(s_vcp, 1)

        @block.vector
        def _(vector):
            vector.wait_ge(s_ld[LD_IO], 16)
            vector.wait_ge(s_ld[LD_RL0], 16)
            kk = 0
            for q in range(NB):
                if q == 2:
                    vector.wait_ge(s_ld[LD_RL1], 16)
                for i, (j, np_, rlb) in enumerate(q_range(q)):
                    for p in range(np_):
                        if kk >= S_BUFS:
                            vector.wait_ge(s_smm, kk - S_BUFS + 1)
                        slot = rlb + p
                        vector.tensor_scalar(
                            s_sb[:, kk % S_BUFS, :],
                            iota_sb[:, :],
                            rl_sb[:, slot:slot + 1],
                            None,
                            mybir.AluOpType.is_equal,
                        ).then_inc(s_s, 1)
                        kk += 1
                if q >= 1:
                    dve_copies(vector, q - 1)
            dve_copies(vector, NB - 1)

        @block.scalar
        def _(scalar):
            for q in range(NB):
                scalar.wait_ge(s_omm, q + 1)
                if q >= 2:
                    scalar.wait_ge(s_ow[q % 2], 16 * (q // 2))
                scalar.activation(
                    o_sb[:, q % 2, :],
                    po[q % 2][:, 0:128],
                    mybir.ActivationFunctionType.Relu,
                ).then_inc(s_ocp, 1)

    nc.compile()
    return nc


def _run(x, edge_index, weight, bias, trace=False):
    x16, w_t, bias_bc, ident, iota16, col16, rloc16, meta, pos = _host_prep(
        x, edge_index, weight, bias
    )
    nc = _build_program(meta)
    in_maps = [
        {
            "x16": x16,
            "w": w_t,
            "bb": bias_bc,
            "ident": ident,
            "iota": iota16,
            "col": np.ascontiguousarray(col16[c]),
            "rl": np.ascontiguousarray(rloc16[c]),
        }
        for c in range(NCORES)
    ]
    res = run_bass_kernel_spmd(nc, in_maps, list(range(NCORES)), trace=trace)
    out = np.concatenate([res.results[c]["out"] for c in range(NCORES)], axis=0)
    return np.ascontiguousarray(out[pos].astype(np.float32)), res


def kernel(x, edge_index, weight, bias):
    out, _ = _run(x, edge_index, weight, bias, trace=False)
    return out


# revision 38
# speedup vs baseline: 1.0060x; 1.0060x over previous
"""GNN message-passing (graph convolution) kernel for 8 Trainium2 NeuronCores.

    out = relu(segment_sum(h[col], row) + bias),  h = x @ W

Strategy (v5, "aggregate-x-then-matmul" + chained dedup): by linearity,
segment_sum(x@W [col], row) = segment_sum(x[col], row) @ W, so the dense
projection is applied AFTER aggregation and the per-edge work is pure data
movement:

  * Host LPT-balances the 20000 nodes into 157 dst blocks of 128 (by degree);
    blocks are assigned contiguously to cores (20/core), so each core
    produces a disjoint output slice - no collectives.
  * Each core's blocks are greedily ordered into a chain maximizing
    consecutive source overlap, and the core consumes ONE gather stream of
    CHAINED per-block segments
    [seg_0 | seg_1 | ... | seg_19]: a segment holds the distinct source
    columns of its block not already emitted in the previous segment, with
    columns also used by the NEXT block placed in a shared tail.  Block q's
    one-hot matmul range covers seg q plus the shared tail of seg q-1, so
    every shared column is gathered once but aggregated into both blocks
    (~20% fewer gathered rows than raw edges).  Only columns with a single
    destination in the current block are shared (keeps its pass schedule
    flat); sources with
    k>=2 destinations inside a block run k one-hot passes and are sorted to
    the front of their segment (per-chunk pass counts are global maxima, so
    the SPMD instruction stream is identical on every core).
  * The gather stream is FLAT: SWDGE dma_gather instructions of 7 chunks
    (896 rows, the ring limit) are issued back to back into an 84-chunk val
    ring, crossing segment boundaries, minimizing the count of gather
    instructions (994ns fixed cost each on the GpSimd engine).  x rows
    (512B fp16) are gathered straight from DRAM.
  * DVE builds one-hot tiles S[e,n] = (iota == rowloc[pass]) in fp16; the PE
    accumulates aggT[f,n] += val[:,c,f]^T @ S over the block's range in PSUM
    fp32 - an exact transposed segment-sum (no PE transposes).
  * DVE copies aggT to SBUF fp16; PE computes out_b = aggT^T @ W + bias
    (bias via an identity-matmul against a broadcast bias tile); ACT applies
    ReLU (fp16 out); DMA out; host re-permutes and upcasts.

Numerics: fp16 operands with fp32 accumulation; one-hot matmuls are exact, so
the only error is fp16 rounding of x, W and the aggregate (~1e-3 relative).
"""

import sys

import numpy as np

sys.path.insert(0, "/opt/trn_rl_repo")

import concourse.bacc as bacc  # noqa: E402
import concourse.bass as bass  # noqa: E402  (engine types)
import concourse.mybir as mybir  # noqa: E402
from concourse.bass_utils import run_bass_kernel_spmd  # noqa: E402

N_NODES = 20000
FIN = 256
FOUT = 128
N_EDGES = 640000

NBLK = 157               # dst blocks of 128 nodes (157*128 = 20096 slots)
NCORES = 8
NB = 20                  # block slots per core (core 7: 17 real + 3 dummy)
NPAIR = NB // 2

S_BUFS = 8               # one-hot tile ring
GRP = 7                  # chunks per gather (896 rows <= 1024-desc SWDGE ring)
VC = 84                  # val ring chunks (multiple of GRP)
NSEM = VC // GRP         # rotating gather-completion semaphores

FP16 = mybir.dt.float16
FP32 = mybir.dt.float32
I16 = mybir.dt.int16


def _host_prep(x, edge_index, weight, bias):
    """Cast operands, balance nodes into blocks, build chained gather streams."""
    import heapq

    x16 = np.ascontiguousarray(np.asarray(x, np.float32).astype(np.float16))
    weight = np.asarray(weight, np.float32)
    bias = np.asarray(bias, np.float32)

    w_t = np.ascontiguousarray(weight.astype(np.float16).reshape(2, 128, 128))
    bias_bc = np.ascontiguousarray(
        np.broadcast_to(bias.astype(np.float16), (128, 128))
    )
    ident = np.eye(128, dtype=np.float16)
    iota16 = np.ascontiguousarray(
        np.broadcast_to(np.arange(128, dtype=np.float16), (128, 128))
    )

    row = np.asarray(edge_index[0]).astype(np.int64)
    col = np.asarray(edge_index[1]).astype(np.int64)

    # LPT-balance nodes into NBLK blocks of <=128 nodes (by degree) to
    # minimize the max edges-per-block.
    deg = np.bincount(row, minlength=N_NODES)
    order = np.argsort(-deg, kind="stable")
    blk_of = np.empty(N_NODES, np.int32)
    slot_of = np.empty(N_NODES, np.int32)
    heap = [(0, b) for b in range(NBLK)]
    heapq.heapify(heap)
    nslots = np.zeros(NBLK, np.int32)
    for n in order:
        load, b = heapq.heappop(heap)
        blk_of[n] = b
        slot_of[n] = nslots[b]
        nslots[b] += 1
        if nslots[b] < 128:
            heapq.heappush(heap, (load + int(deg[n]), b))

    b_of_edge = blk_of[row]
    eorder = np.argsort(b_of_edge, kind="stable")
    cs = col[eorder].astype(np.int32)
    rloc = slot_of[row[eorder]].astype(np.int32)
    counts = np.bincount(b_of_edge, minlength=NBLK)
    starts = np.concatenate([[0], np.cumsum(counts)])

    # Build one dedup'd CHAIN stream per core: segments seg_0..seg_19, one
    # per block slot.  A column of block q already emitted in seg q-1 (and
    # thus covered by range q) is not re-emitted; emitted columns also in
    # block q+1 form the segment's "shared" tail, consumed by range q+1 too.
    def blk_edges(g):
        if g < 0 or g >= NBLK:
            return (np.empty(0, np.int32), np.empty(0, np.int32))
        e0, e1 = int(starts[g]), int(starts[g + 1])
        return cs[e0:e1], rloc[e0:e1]

    # Pre-group every block's edges by column.
    ublk = []
    for g in range(NBLK):
        c_b, r_b = blk_edges(g)
        o2 = np.argsort(c_b, kind="stable")
        sc, sr = c_b[o2], r_b[o2]
        new = np.r_[True, sc[1:] != sc[:-1]] if sc.size else np.empty(0, bool)
        first = np.flatnonzero(new)
        gid = np.cumsum(new) - 1 if sc.size else np.empty(0, np.int64)
        dcnt = np.diff(np.r_[first, sc.size]) if sc.size else np.empty(0, np.int64)
        rank = (np.arange(sc.size) - first[gid]) if sc.size else np.empty(0, np.int64)
        ublk.append((sc[first] if sc.size else np.empty(0, np.int32),
                     dcnt, gid, rank, sr))

    # Order each core's blocks into a chain that maximizes consecutive
    # source overlap (greedy path extension on |U_a cap U_b|).
    chain = np.full((NCORES, NB), -1, np.int64)
    for c in range(NCORES):
        ids = list(range(c * NB, min((c + 1) * NB, NBLK)))
        n = len(ids)
        ov = np.zeros((n, n), np.int64)
        for a in range(n):
            for b in range(a + 1, n):
                w = np.intersect1d(
                    ublk[ids[a]][0], ublk[ids[b]][0], assume_unique=True
                ).size
                ov[a, b] = ov[b, a] = w
        a0, b0 = divmod(int(np.argmax(ov)), n)
        path = [a0, b0]
        used = {a0, b0}
        while len(path) < n:
            rest = [k for k in range(n) if k not in used]
            headw = max(rest, key=lambda k: ov[path[0], k])
            tailw = max(rest, key=lambda k: ov[path[-1], k])
            if ov[path[0], headw] > ov[path[-1], tailw]:
                path.insert(0, headw)
                used.add(headw)
            else:
                path.append(tailw)
                used.add(tailw)
        chain[c, :n] = [ids[k] for k in path]
    # relabel blocks to chain position so the output layout follows
    newpos = np.full(NBLK, -1, np.int64)
    for c in range(NCORES):
        for q in range(NB):
            if chain[c, q] >= 0:
                newpos[chain[c, q]] = c * NB + q

    E = np.empty(0, np.int64)
    seg_n = np.zeros((NCORES, NB), np.int64)    # rows per segment
    seg_only = np.zeros((NCORES, NB), np.int64)  # rows before shared tail
    seg_cols = {}                                # (c,q) -> col values in order
    entries = {}   # (c,q) -> (seg_of_row, u=row//128, part, rank, rl)
    for c in range(NCORES):
        prev_cols = np.empty(0, np.int32)        # emitted shared cols of q-1
        prev_pos = np.empty(0, np.int64)         # their row pos in seg q-1
        for q in range(NB):
            g = int(chain[c, q])
            if g < 0:
                seg_cols[(c, q)] = np.empty(0, np.int32)
                entries[(c, q)] = (E, E, E, E, E)
                prev_cols, prev_pos = np.empty(0, np.int32), E
                continue
            ucols, dcnt, gid, rank, sr = ublk[g]
            ng = ucols.size
            covered = np.isin(ucols, prev_cols)
            emit_idx = np.flatnonzero(~covered)
            emit_cols = ucols[emit_idx]
            gn = int(chain[c, q + 1]) if q + 1 < NB else -1
            if gn >= 0:
                nxt_ucols, nxt_dcnt = ublk[gn][0], ublk[gn][1]
                ip = np.searchsorted(nxt_ucols, emit_cols)
                ip = np.minimum(ip, max(nxt_ucols.size - 1, 0))
                in_next = (nxt_ucols.size > 0) & (nxt_ucols[ip] == emit_cols)
                d_next = np.where(in_next, nxt_dcnt[ip], 0)
            else:
                in_next = np.zeros(emit_cols.size, bool)
                d_next = np.zeros(emit_cols.size, np.int64)
            d_cur = dcnt[emit_idx]
            # share only columns with a single dst in the CURRENT block: the
            # shared tail stays flat (1 pass) for range q; multi-dst-in-next
            # columns only inflate the head of range q+1 (sorted desc there)
            in_next = in_next & (d_cur == 1)
            key = np.where(in_next, np.maximum(d_cur, d_next), d_cur)
            uo = np.lexsort((-key, in_next.astype(np.int8)))
            pos_of_emit = np.empty(emit_cols.size, np.int64)
            pos_of_emit[uo] = np.arange(emit_cols.size)
            seg_cols[(c, q)] = emit_cols[uo]
            seg_n[c, q] = emit_cols.size
            seg_only[c, q] = int((~in_next).sum())
            # per-group row position: covered -> prev seg, else this seg
            grow = np.empty(ng, np.int64)
            gseg = np.empty(ng, np.int64)
            if prev_cols.size:
                pi = np.searchsorted(prev_cols, ucols[covered])
                grow[covered] = prev_pos[pi]
                gseg[covered] = q - 1
            grow[~covered] = pos_of_emit
            gseg[~covered] = q
            entries[(c, q)] = (
                gseg[gid], grow[gid] // 128, grow[gid] % 128,
                rank, sr.astype(np.int64),
            )
            shared = np.flatnonzero(in_next[uo])
            prev_cols = emit_cols[uo][shared]
            po = np.argsort(prev_cols, kind="stable")
            prev_cols = prev_cols[po]
            prev_pos = shared[po]

    segc = np.maximum(seg_n.max(axis=0) + 127, 128) // 128  # chunks per seg
    posq = np.concatenate([[0], np.cumsum(segc)])
    tc = int(posq[-1])                                       # chunks per core
    rs, re = [0] * NB, [0] * NB
    for q in range(NB):
        re[q] = int(posq[q] + segc[q])
        if q == 0:
            rs[q] = 0
        else:
            live = seg_n[:, q - 1] > 0
            off = int((seg_only[live, q - 1] // 128).min()) if live.any() else \
                int(segc[q - 1])
            rs[q] = int(posq[q - 1]) + off

    # Static pass schedules: per-range per-chunk max of dst-counts.
    passes = []
    for q in range(NB):
        pq = np.zeros(re[q] - rs[q], np.int64)
        for c in range(NCORES):
            gseg, u, part, rank, rl = entries[(c, q)]
            if len(gseg) == 0:
                continue
            i = posq[gseg] + u - rs[q]
            np.maximum.at(pq, i, rank + 1)
        passes.append(np.maximum(pq, 1))
    pcums = [np.concatenate([[0], np.cumsum(p)]) for p in passes]
    sbase = np.concatenate([[0], np.cumsum([int(p[-1]) for p in pcums])])
    tslots = int(sbase[-1])

    nidx = tc * 128
    idxc = nidx // 16
    col16 = np.zeros((NCORES, 128, idxc), np.int16)
    rloc16 = np.full((NCORES, 128, tslots), -1.0, np.float32)
    for c in range(NCORES):
        lin_col = np.zeros(nidx, np.int32)
        lin_rl = np.full((tslots, 128), -1.0, np.float32)
        for q in range(NB):
            u0 = seg_cols[(c, q)]
            lin_col[posq[q] * 128:posq[q] * 128 + u0.size] = u0
            gseg, u, part, rank, rl = entries[(c, q)]
            if len(gseg) == 0:
                continue
            i = posq[gseg] + u - rs[q]
            lin_rl[sbase[q] + pcums[q][i] + rank, part] = rl
        # the SWDGE Q7 cores read the indices from different 16-partition
        # groups - replicate the 16-row wrap to all 128
        col16[c] = np.tile(lin_col.reshape(idxc, 16).T.astype(np.int16), (8, 1))
        rloc16[c] = lin_rl.T
    # out_concat[blk*128 + slot] -> node (block ids are already slot order)
    pos = (newpos[blk_of] * 128 + slot_of).astype(np.int64)
    meta = (tc, rs, re, [list(map(int, p)) for p in passes])
    return x16, w_t, bias_bc, ident, iota16, col16, rloc16, meta, pos


def _build_program(meta):
    tc, rs, re, passes = meta
    pcums = []
    for p in passes:
        c = [0]
        for v in p:
            c.append(c[-1] + v)
        pcums.append(c)
    sbase = [0]
    for c in pcums:
        sbase.append(sbase[-1] + c[-1])
    tslots = sbase[-1]
    idxc = tc * 8
    nch = tc                         # global chunk count
    ngat = (nch + GRP - 1) // GRP    # flat gather instructions

    # range of block-slot q: list of (chunk j, passes, rl slot base)
    def q_range(q):
        return [
            (rs[q] + i, passes[q][i], sbase[q] + pcums[q][i])
            for i in range(re[q] - rs[q])
        ]

    def smm_after(q):  # s_smm value after block-slot q's range completes
        return sbase[q + 1]

    # s_smm value at which chunk j is fully consumed (last covering range)
    tgt = [0] * nch
    for q in range(NB):
        for i in range(re[q] - rs[q]):
            tgt[rs[q] + i] = sbase[q] + pcums[q][i + 1]

    def consume_tgt(j):
        return tgt[j]

    # ramp split points: chunks/slots of the first two ranges
    ch0 = re[1]
    sl0 = sbase[2]

    nc = bacc.Bacc("TRN2")

    x_d = nc.dram_tensor("x16", [N_NODES, FIN], FP16, kind="ExternalInput")
    w_d = nc.dram_tensor("w", [2, 128, 128], FP16, kind="ExternalInput")
    bb_d = nc.dram_tensor("bb", [128, 128], FP16, kind="ExternalInput")
    id_d = nc.dram_tensor("ident", [128, 128], FP16, kind="ExternalInput")
    io_d = nc.dram_tensor("iota", [128, 128], FP16, kind="ExternalInput")
    col_d = nc.dram_tensor("col", [128, idxc], I16, kind="ExternalInput")
    rl_d = nc.dram_tensor("rl", [128, tslots], FP32, kind="ExternalInput")
    o_d = nc.dram_tensor("out", [NB * 128, 128], FP16, kind="ExternalOutput")

    from contextlib import ExitStack

    with ExitStack() as es:
        # aggT accumulators: [parity][feature-half], one bank each
        pa = [
            [es.enter_context(nc.psum_tensor(f"pa{k}{h}", [128, 512], FP32))
             for h in range(2)]
            for k in range(2)
        ]
        po = [es.enter_context(nc.psum_tensor(f"po{k}", [128, 512], FP32))
              for k in range(2)]
        w_sb = es.enter_context(nc.sbuf_tensor("w_sb", [128, 2, 128], FP16))
        bb_sb = es.enter_context(nc.sbuf_tensor("bb_sb", [128, 128], FP16))
        id_sb = es.enter_context(nc.sbuf_tensor("id_sb", [128, 128], FP16))
        iota_sb = es.enter_context(nc.sbuf_tensor("iota_sb", [128, 128], FP16))
        col_sb = es.enter_context(
            nc.sbuf_tensor("col_sb", [128, idxc], I16)
        )
        rl_sb = es.enter_context(
            nc.sbuf_tensor("rl_sb", [128, tslots], FP32)
        )
        val_sb = es.enter_context(nc.sbuf_tensor("val_sb", [128, VC, FIN], FP16))
        s_sb = es.enter_context(nc.sbuf_tensor("s_sb", [128, S_BUFS, 128], FP16))
        at_sb = es.enter_context(nc.sbuf_tensor("at_sb", [128, 2, 2, 128], FP16))
        o_sb = es.enter_context(nc.sbuf_tensor("o_sb", [128, 2, 128], FP16))

        s_ld = [es.enter_context(nc.semaphore(f"s_ld{k}")) for k in range(10)]
        s_gat = [
            es.enter_context(nc.semaphore(f"s_gat{k}")) for k in range(NSEM)
        ]
        s_ow = [es.enter_context(nc.semaphore(f"s_ow{k}")) for k in range(2)]
        s_s = es.enter_context(nc.semaphore("s_s"))      # DVE one-hot count
        s_smm = es.enter_context(nc.semaphore("s_smm"))  # PE pass-mm count
        s_vcp = es.enter_context(nc.semaphore("s_vcp"))  # DVE aggT copies
        s_omm = es.enter_context(nc.semaphore("s_omm"))  # PE final-mm count
        s_ocp = es.enter_context(nc.semaphore("s_ocp"))  # ACT relu count
        block = es.enter_context(nc.Block())

        (LD_COL00, LD_COL0, LD_COL1, LD_IO, LD_RL0, LD_RL1, LD_W, LD_W1,
         LD_BB, LD_ID) = range(10)

        @block.sync
        def _(sync):
            # Ramp-critical loads first: pair-0 idx slice gates the first
            # gather; iota + pair-0 rloc gate the first one-hot builds.
            sync.dma_start(iota_sb[:, :], io_d[:, :]).then_inc(s_ld[LD_IO], 16)
            sync.dma_start(
                rl_sb[:, 0:sl0], rl_d[:, 0:sl0]
            ).then_inc(s_ld[LD_RL0], 16)
            sync.dma_start(
                col_sb[:, ch0 * 8:], col_d[:, ch0 * 8:]
            ).then_inc(s_ld[LD_COL1], 16)
            sync.dma_start(
                rl_sb[:, sl0:], rl_d[:, sl0:]
            ).then_inc(s_ld[LD_RL1], 16)
            sync.dma_start(w_sb[:, 0, :], w_d[0]).then_inc(s_ld[LD_W], 16)
            sync.dma_start(w_sb[:, 1, :], w_d[1]).then_inc(s_ld[LD_W1], 16)
            sync.dma_start(bb_sb[:, :], bb_d[:, :]).then_inc(s_ld[LD_BB], 16)
            sync.dma_start(id_sb[:, :], id_d[:, :]).then_inc(s_ld[LD_ID], 16)
            for b in range(NB):
                sync.wait_ge(s_ocp, b + 1)
                sync.dma_start(
                    o_d[b * 128:(b + 1) * 128, :], o_sb[:, b % 2, :]
                ).then_inc(s_ow[b % 2], 16)

        @block.gpsimd
        def _(gpsimd):
            # self-load the ramp-critical idx slice via mainline SWDGE: no
            # cross-engine hop, ~0.8us earlier first gather
            gpsimd.dma_start(
                col_sb[:, 0:ch0 * 8], col_d[:, 0:ch0 * 8]
            ).then_inc(s_ld[LD_COL0], 16)
            gpsimd.wait_ge(s_ld[LD_COL0], 16)
            g_cross = next(g for g in range(ngat + 1) if GRP * g + GRP > ch0)
            for g in range(ngat):
                if g == g_cross:
                    gpsimd.wait_ge(s_ld[LD_COL1], 16)
                j0, j1 = GRP * g, min(GRP * g + GRP, nch)
                if j1 - VC > 0:
                    # val ring slots [j0%VC, ...) held chunks [j0-VC, j1-VC)
                    gpsimd.wait_ge(s_smm, consume_tgt(j1 - VC - 1))
                r = j0 % VC
                gpsimd.dma_gather(
                    val_sb[:, r:r + (j1 - j0), :],
                    x_d[:, :],
                    col_sb[:, j0 * 8:j1 * 8],
                    (j1 - j0) * 128,
                    (j1 - j0) * 128,
                    FIN,
                ).then_inc(s_gat[g % NSEM], 16)

        def pe_final(tensor, q):
            if q == 0:
                for k in (LD_W, LD_W1, LD_BB, LD_ID):
                    tensor.wait_ge(s_ld[k], 16)
            tensor.wait_ge(s_vcp, 2 * (q + 1))
            if q >= 2:
                tensor.wait_ge(s_ocp, q - 1)
            tensor.matmul(
                po[q % 2][:, 0:128], id_sb[:, :], bb_sb[:, :],
                start=True, stop=False,
            )
            tensor.matmul(
                po[q % 2][:, 0:128], at_sb[:, q % 2, 0, :], w_sb[:, 0, :],
                start=False, stop=False,
            )
            tensor.matmul(
                po[q % 2][:, 0:128], at_sb[:, q % 2, 1, :], w_sb[:, 1, :],
                start=False, stop=True,
            ).then_inc(s_omm, 1)

        @block.tensor
        def _(tensor):
            kk = 0
            waited_g = 0
            for q in range(NB):
                rng = q_range(q)
                for i, (j, np_, rlb) in enumerate(rng):
                    g = j // GRP
                    while waited_g <= g:
                        tensor.wait_ge(
                            s_gat[waited_g % NSEM],
                            16 * (waited_g // NSEM + 1),
                        )
                        waited_g += 1
                    if i == 0 and q >= 2:
                        # pa[q%2] fully copied out (block-slot q-2)
                        tensor.wait_ge(s_vcp, 2 * (q - 1))
                    for p in range(np_):
                        tensor.wait_ge(s_s, kk + 1)
                        st = i == 0 and p == 0
                        sp = i == len(rng) - 1 and p == np_ - 1
                        tensor.matmul(
                            pa[q % 2][0][:, 0:128],
                            val_sb[:, j % VC, 0:128],
                            s_sb[:, kk % S_BUFS, :],
                            start=st,
                            stop=sp,
                        )
                        tensor.matmul(
                            pa[q % 2][1][:, 0:128],
                            val_sb[:, j % VC, 128:256],
                            s_sb[:, kk % S_BUFS, :],
                            start=st,
                            stop=sp,
                        ).then_inc(s_smm, 1)
                        kk += 1
                if q >= 1:
                    pe_final(tensor, q - 1)
            pe_final(tensor, NB - 1)

        def dve_copies(vector, q):
            vector.wait_ge(s_smm, smm_after(q))
            if q >= 2:
                # at_sb[q%2] consumed by pe_final(q-2)
                vector.wait_ge(s_omm, q - 1)
            vector.tensor_copy(
                at_sb[:, q % 2, 0, :], pa[q % 2][0][:, 0:128]
            ).then_inc(s_vcp, 1)
            vector.tensor_copy(
                at_sb[:, q % 2, 1, :], pa[q % 2][1][:, 0:128]
            ).then_inc(s_vcp, 1)

        @block.vector
        def _(vector):
            vector.wait_ge(s_ld[LD_IO], 16)
            vector.wait_ge(s_ld[LD_RL0], 16)
            kk = 0
            for q in range(NB):
                if q == 2:
                    vector.wait_ge(s_ld[LD_RL1], 16)
                for i, (j, np_, rlb) in enumerate(q_range(q)):
                    for p in range(np_):
                        if kk >= S_BUFS:
                            vector.wait_ge(s_smm, kk - S_BUFS + 1)
                        slot = rlb + p
                        vector.tensor_scalar(
                            s_sb[:, kk % S_BUFS, :],
                            iota_sb[:, :],
                            rl_sb[:, slot:slot + 1],
                            None,
                            mybir.AluOpType.is_equal,
                        ).then_inc(s_s, 1)
                        kk += 1
                if q >= 1:
                    dve_copies(vector, q - 1)
            dve_copies(vector, NB - 1)

        @block.scalar
        def _(scalar):
            for q in range(NB):
                scalar.wait_ge(s_omm, q + 1)
                if q >= 2:
                    scalar.wait_ge(s_ow[q % 2], 16 * (q // 2))
                scalar.activation(
                    o_sb[:, q % 2, :],
                    po[q % 2][:, 0:128],
                    mybir.ActivationFunctionType.Relu,
                ).then_inc(s_ocp, 1)

    nc.compile()
    return nc


def _run(x, edge_index, weight, bias, trace=False):
    x16, w_t, bias_bc, ident, iota16, col16, rloc16, meta, pos = _host_prep(
        x, edge_index, weight, bias
    )
    nc = _build_program(meta)
    in_maps = [
        {
            "x16": x16,
            "w": w_t,
            "bb": bias_bc,
            "ident": ident,
            "iota": iota16,
            "col": np.ascontiguousarray(col16[c]),
            "rl": np.ascontiguousarray(rloc16[c]),
        }
        for c in range(NCORES)
    ]
    res = run_bass_kernel_spmd(nc, in_maps, list(range(NCORES)), trace=trace)
    out = np.concatenate([res.results[c]["out"] for c in range(NCORES)], axis=0)
    return np.ascontiguousarray(out[pos].astype(np.float32)), res


def kernel(x, edge_index, weight, bias):
    out, _ = _run(x, edge_index, weight, bias, trace=False)
    return out


# revision 39
# speedup vs baseline: 1.0228x; 1.0167x over previous
"""GNN message-passing (graph convolution) kernel for 8 Trainium2 NeuronCores.

    out = relu(segment_sum(h[col], row) + bias),  h = x @ W

Strategy (v5, "aggregate-x-then-matmul" + chained dedup): by linearity,
segment_sum(x@W [col], row) = segment_sum(x[col], row) @ W, so the dense
projection is applied AFTER aggregation and the per-edge work is pure data
movement:

  * Host LPT-balances the 20000 nodes into 157 dst blocks of 128 (by degree);
    blocks are assigned contiguously to cores (20/core), so each core
    produces a disjoint output slice - no collectives.
  * Each core's blocks are greedily ordered into a chain maximizing
    consecutive source overlap, and the core consumes ONE gather stream of
    CHAINED per-block segments
    [seg_0 | seg_1 | ... | seg_19]: a segment holds the distinct source
    columns of its block not already emitted in the previous segment, with
    columns also used by the NEXT block placed in a shared tail.  Block q's
    one-hot matmul range covers seg q plus the shared tail of seg q-1, so
    every shared column is gathered once but aggregated into both blocks
    (~20% fewer gathered rows than raw edges).  Only columns with a single
    destination in the current block are shared (keeps its pass schedule
    flat); sources with
    k>=2 destinations inside a block run k one-hot passes and are sorted to
    the front of their segment (per-chunk pass counts are global maxima, so
    the SPMD instruction stream is identical on every core).
  * The gather stream is FLAT: SWDGE dma_gather instructions of 7 chunks
    (896 rows, the ring limit) are issued back to back into an 84-chunk val
    ring, crossing segment boundaries, minimizing the count of gather
    instructions (994ns fixed cost each on the GpSimd engine).  x rows
    (512B fp16) are gathered straight from DRAM.
  * DVE builds one-hot tiles S[e,n] = (iota == rowloc[pass]) in fp16; the PE
    accumulates aggT[f,n] += val[:,c,f]^T @ S over the block's range in PSUM
    fp32 - an exact transposed segment-sum (no PE transposes).
  * DVE copies aggT to SBUF fp16; PE computes out_b = aggT^T @ W + bias
    (bias via an identity-matmul against a broadcast bias tile); ACT applies
    ReLU (fp16 out); DMA out; host re-permutes and upcasts.

Numerics: fp16 operands with fp32 accumulation; one-hot matmuls are exact, so
the only error is fp16 rounding of x, W and the aggregate (~1e-3 relative).
"""

import sys

import numpy as np

sys.path.insert(0, "/opt/trn_rl_repo")

import concourse.bacc as bacc  # noqa: E402
import concourse.bass as bass  # noqa: E402  (engine types)
import concourse.mybir as mybir  # noqa: E402
from concourse.bass_utils import run_bass_kernel_spmd  # noqa: E402

N_NODES = 20000
FIN = 256
FOUT = 128
N_EDGES = 640000

NBLK = 157               # dst blocks of 128 nodes (157*128 = 20096 slots)
NCORES = 8
NB = 20                  # block slots per core (core 7: 17 real + 3 dummy)
NPAIR = NB // 2

S_BUFS = 8               # one-hot tile ring
GRP = 7                  # chunks per gather (896 rows <= 1024-desc SWDGE ring)
VC = 84                  # val ring chunks (multiple of GRP)
NSEM = VC // GRP         # rotating gather-completion semaphores

FP16 = mybir.dt.float16
FP32 = mybir.dt.float32
I16 = mybir.dt.int16


def _host_prep(x, edge_index, weight, bias):
    """Cast operands, balance nodes into blocks, build chained gather streams."""
    import heapq

    x16 = np.ascontiguousarray(np.asarray(x, np.float32).astype(np.float16))
    weight = np.asarray(weight, np.float32)
    bias = np.asarray(bias, np.float32)

    w_t = np.ascontiguousarray(weight.astype(np.float16).reshape(2, 128, 128))
    bias_bc = np.ascontiguousarray(
        np.broadcast_to(bias.astype(np.float16), (128, 128))
    )
    ident = np.eye(128, dtype=np.float16)
    iota16 = np.ascontiguousarray(
        np.broadcast_to(np.arange(128, dtype=np.float16), (128, 128))
    )

    row = np.asarray(edge_index[0]).astype(np.int64)
    col = np.asarray(edge_index[1]).astype(np.int64)

    # LPT-balance nodes into NBLK blocks of <=128 nodes (by degree) to
    # minimize the max edges-per-block.
    deg = np.bincount(row, minlength=N_NODES)
    order = np.argsort(-deg, kind="stable")
    blk_of = np.empty(N_NODES, np.int32)
    slot_of = np.empty(N_NODES, np.int32)
    heap = [(0, b) for b in range(NBLK)]
    heapq.heapify(heap)
    nslots = np.zeros(NBLK, np.int32)
    for n in order:
        load, b = heapq.heappop(heap)
        blk_of[n] = b
        slot_of[n] = nslots[b]
        nslots[b] += 1
        if nslots[b] < 128:
            heapq.heappush(heap, (load + int(deg[n]), b))

    b_of_edge = blk_of[row]
    eorder = np.argsort(b_of_edge, kind="stable")
    cs = col[eorder].astype(np.int32)
    rloc = slot_of[row[eorder]].astype(np.int32)
    counts = np.bincount(b_of_edge, minlength=NBLK)
    starts = np.concatenate([[0], np.cumsum(counts)])

    # Build one dedup'd CHAIN stream per core: segments seg_0..seg_19, one
    # per block slot.  A column of block q already emitted in seg q-1 (and
    # thus covered by range q) is not re-emitted; emitted columns also in
    # block q+1 form the segment's "shared" tail, consumed by range q+1 too.
    def blk_edges(g):
        if g < 0 or g >= NBLK:
            return (np.empty(0, np.int32), np.empty(0, np.int32))
        e0, e1 = int(starts[g]), int(starts[g + 1])
        return cs[e0:e1], rloc[e0:e1]

    # Pre-group every block's edges by column.
    ublk = []
    for g in range(NBLK):
        c_b, r_b = blk_edges(g)
        o2 = np.argsort(c_b, kind="stable")
        sc, sr = c_b[o2], r_b[o2]
        new = np.r_[True, sc[1:] != sc[:-1]] if sc.size else np.empty(0, bool)
        first = np.flatnonzero(new)
        gid = np.cumsum(new) - 1 if sc.size else np.empty(0, np.int64)
        dcnt = np.diff(np.r_[first, sc.size]) if sc.size else np.empty(0, np.int64)
        rank = (np.arange(sc.size) - first[gid]) if sc.size else np.empty(0, np.int64)
        ublk.append((sc[first] if sc.size else np.empty(0, np.int32),
                     dcnt, gid, rank, sr))

    # Order each core's blocks into a chain that maximizes consecutive
    # source overlap (greedy path extension on |U_a cap U_b|).
    chain = np.full((NCORES, NB), -1, np.int64)
    for c in range(NCORES):
        ids = list(range(c * NB, min((c + 1) * NB, NBLK)))
        n = len(ids)
        ov = np.zeros((n, n), np.int64)
        for a in range(n):
            for b in range(a + 1, n):
                w = np.intersect1d(
                    ublk[ids[a]][0], ublk[ids[b]][0], assume_unique=True
                ).size
                ov[a, b] = ov[b, a] = w
        a0, b0 = divmod(int(np.argmax(ov)), n)
        path = [a0, b0]
        used = {a0, b0}
        while len(path) < n:
            rest = [k for k in range(n) if k not in used]
            headw = max(rest, key=lambda k: ov[path[0], k])
            tailw = max(rest, key=lambda k: ov[path[-1], k])
            if ov[path[0], headw] > ov[path[-1], tailw]:
                path.insert(0, headw)
                used.add(headw)
            else:
                path.append(tailw)
                used.add(tailw)
        chain[c, :n] = [ids[k] for k in path]
    # relabel blocks to chain position so the output layout follows
    newpos = np.full(NBLK, -1, np.int64)
    for c in range(NCORES):
        for q in range(NB):
            if chain[c, q] >= 0:
                newpos[chain[c, q]] = c * NB + q

    E = np.empty(0, np.int64)
    seg_n = np.zeros((NCORES, NB), np.int64)    # rows per segment
    seg_only = np.zeros((NCORES, NB), np.int64)  # rows before shared tail
    seg_cols = {}                                # (c,q) -> col values in order
    entries = {}   # (c,q) -> (seg_of_row, u=row//128, part, rank, rl)
    for c in range(NCORES):
        prev_cols = np.empty(0, np.int32)        # emitted shared cols of q-1
        prev_pos = np.empty(0, np.int64)         # their row pos in seg q-1
        for q in range(NB):
            g = int(chain[c, q])
            if g < 0:
                seg_cols[(c, q)] = np.empty(0, np.int32)
                entries[(c, q)] = (E, E, E, E, E)
                prev_cols, prev_pos = np.empty(0, np.int32), E
                continue
            ucols, dcnt, gid, rank, sr = ublk[g]
            ng = ucols.size
            covered = np.isin(ucols, prev_cols)
            emit_idx = np.flatnonzero(~covered)
            emit_cols = ucols[emit_idx]
            gn = int(chain[c, q + 1]) if q + 1 < NB else -1
            if gn >= 0:
                nxt_ucols, nxt_dcnt = ublk[gn][0], ublk[gn][1]
                ip = np.searchsorted(nxt_ucols, emit_cols)
                ip = np.minimum(ip, max(nxt_ucols.size - 1, 0))
                in_next = (nxt_ucols.size > 0) & (nxt_ucols[ip] == emit_cols)
                d_next = np.where(in_next, nxt_dcnt[ip], 0)
            else:
                in_next = np.zeros(emit_cols.size, bool)
                d_next = np.zeros(emit_cols.size, np.int64)
            d_cur = dcnt[emit_idx]
            # share only columns with a single dst in the CURRENT block: the
            # shared tail stays flat (1 pass) for range q; multi-dst-in-next
            # columns only inflate the head of range q+1 (sorted desc there)
            in_next = in_next & (d_cur == 1)
            key = np.where(in_next, np.maximum(d_cur, d_next), d_cur)
            uo = np.lexsort((-key, in_next.astype(np.int8)))
            pos_of_emit = np.empty(emit_cols.size, np.int64)
            pos_of_emit[uo] = np.arange(emit_cols.size)
            seg_cols[(c, q)] = emit_cols[uo]
            seg_n[c, q] = emit_cols.size
            seg_only[c, q] = int((~in_next).sum())
            # per-group row position: covered -> prev seg, else this seg
            grow = np.empty(ng, np.int64)
            gseg = np.empty(ng, np.int64)
            if prev_cols.size:
                pi = np.searchsorted(prev_cols, ucols[covered])
                grow[covered] = prev_pos[pi]
                gseg[covered] = q - 1
            grow[~covered] = pos_of_emit
            gseg[~covered] = q
            entries[(c, q)] = (
                gseg[gid], grow[gid] // 128, grow[gid] % 128,
                rank, sr.astype(np.int64),
            )
            shared = np.flatnonzero(in_next[uo])
            prev_cols = emit_cols[uo][shared]
            po = np.argsort(prev_cols, kind="stable")
            prev_cols = prev_cols[po]
            prev_pos = shared[po]

    segc = np.maximum(seg_n.max(axis=0) + 127, 128) // 128  # chunks per seg
    posq = np.concatenate([[0], np.cumsum(segc)])
    tc = int(posq[-1])                                       # chunks per core
    rs, re = [0] * NB, [0] * NB
    for q in range(NB):
        re[q] = int(posq[q] + segc[q])
        if q == 0:
            rs[q] = 0
        else:
            live = seg_n[:, q - 1] > 0
            off = int((seg_only[live, q - 1] // 128).min()) if live.any() else \
                int(segc[q - 1])
            rs[q] = int(posq[q - 1]) + off

    # Static pass schedules: per-range per-chunk max of dst-counts.
    passes = []
    for q in range(NB):
        pq = np.zeros(re[q] - rs[q], np.int64)
        for c in range(NCORES):
            gseg, u, part, rank, rl = entries[(c, q)]
            if len(gseg) == 0:
                continue
            i = posq[gseg] + u - rs[q]
            np.maximum.at(pq, i, rank + 1)
        passes.append(np.maximum(pq, 1))
    pcums = [np.concatenate([[0], np.cumsum(p)]) for p in passes]
    sbase = np.concatenate([[0], np.cumsum([int(p[-1]) for p in pcums])])
    tslots = int(sbase[-1])

    nidx = tc * 128
    idxc = nidx // 16
    col16 = np.zeros((NCORES, 32, idxc), np.int16)
    rloc16 = np.full((NCORES, 128, tslots), -1.0, np.float32)
    for c in range(NCORES):
        lin_col = np.zeros(nidx, np.int32)
        lin_rl = np.full((tslots, 128), -1.0, np.float32)
        for q in range(NB):
            u0 = seg_cols[(c, q)]
            lin_col[posq[q] * 128:posq[q] * 128 + u0.size] = u0
            gseg, u, part, rank, rl = entries[(c, q)]
            if len(gseg) == 0:
                continue
            i = posq[gseg] + u - rs[q]
            lin_rl[sbase[q] + pcums[q][i] + rank, part] = rl
        # the SWDGE Q7 cores read the indices from different 16-partition
        # groups - replicate the 16-row wrap to all 128
        col16[c] = np.tile(lin_col.reshape(idxc, 16).T.astype(np.int16), (2, 1))
        rloc16[c] = lin_rl.T
    # out_concat[blk*128 + slot] -> node (block ids are already slot order)
    pos = (newpos[blk_of] * 128 + slot_of).astype(np.int64)
    meta = (tc, rs, re, [list(map(int, p)) for p in passes])
    return x16, w_t, bias_bc, ident, iota16, col16, rloc16, meta, pos


def _build_program(meta):
    tc, rs, re, passes = meta
    pcums = []
    for p in passes:
        c = [0]
        for v in p:
            c.append(c[-1] + v)
        pcums.append(c)
    sbase = [0]
    for c in pcums:
        sbase.append(sbase[-1] + c[-1])
    tslots = sbase[-1]
    idxc = tc * 8
    nch = tc                         # global chunk count
    ngat = (nch + GRP - 1) // GRP    # flat gather instructions

    # range of block-slot q: list of (chunk j, passes, rl slot base)
    def q_range(q):
        return [
            (rs[q] + i, passes[q][i], sbase[q] + pcums[q][i])
            for i in range(re[q] - rs[q])
        ]

    def smm_after(q):  # s_smm value after block-slot q's range completes
        return sbase[q + 1]

    # s_smm value at which chunk j is fully consumed (last covering range)
    tgt = [0] * nch
    for q in range(NB):
        for i in range(re[q] - rs[q]):
            tgt[rs[q] + i] = sbase[q] + pcums[q][i + 1]

    def consume_tgt(j):
        return tgt[j]

    # ramp split points: chunks/slots of the first two ranges
    ch0 = re[1]
    sl0 = sbase[2]

    nc = bacc.Bacc("TRN2")

    x_d = nc.dram_tensor("x16", [N_NODES, FIN], FP16, kind="ExternalInput")
    w_d = nc.dram_tensor("w", [2, 128, 128], FP16, kind="ExternalInput")
    bb_d = nc.dram_tensor("bb", [128, 128], FP16, kind="ExternalInput")
    id_d = nc.dram_tensor("ident", [128, 128], FP16, kind="ExternalInput")
    io_d = nc.dram_tensor("iota", [128, 128], FP16, kind="ExternalInput")
    col_d = nc.dram_tensor("col", [32, idxc], I16, kind="ExternalInput")
    rl_d = nc.dram_tensor("rl", [128, tslots], FP32, kind="ExternalInput")
    o_d = nc.dram_tensor("out", [NB * 128, 128], FP16, kind="ExternalOutput")

    from contextlib import ExitStack

    with ExitStack() as es:
        # aggT accumulators: [parity][feature-half], one bank each
        pa = [
            [es.enter_context(nc.psum_tensor(f"pa{k}{h}", [128, 512], FP32))
             for h in range(2)]
            for k in range(2)
        ]
        po = [es.enter_context(nc.psum_tensor(f"po{k}", [128, 512], FP32))
              for k in range(2)]
        w_sb = es.enter_context(nc.sbuf_tensor("w_sb", [128, 2, 128], FP16))
        bb_sb = es.enter_context(nc.sbuf_tensor("bb_sb", [128, 128], FP16))
        id_sb = es.enter_context(nc.sbuf_tensor("id_sb", [128, 128], FP16))
        iota_sb = es.enter_context(nc.sbuf_tensor("iota_sb", [128, 128], FP16))
        col_sb = es.enter_context(
            nc.sbuf_tensor("col_sb", [128, idxc], I16)
        )
        rl_sb = es.enter_context(
            nc.sbuf_tensor("rl_sb", [128, tslots], FP32)
        )
        val_sb = es.enter_context(nc.sbuf_tensor("val_sb", [128, VC, FIN], FP16))
        s_sb = es.enter_context(nc.sbuf_tensor("s_sb", [128, S_BUFS, 128], FP16))
        at_sb = es.enter_context(nc.sbuf_tensor("at_sb", [128, 2, 2, 128], FP16))
        o_sb = es.enter_context(nc.sbuf_tensor("o_sb", [128, 2, 128], FP16))

        s_ld = [es.enter_context(nc.semaphore(f"s_ld{k}")) for k in range(10)]
        s_gat = [
            es.enter_context(nc.semaphore(f"s_gat{k}")) for k in range(NSEM)
        ]
        s_ow = [es.enter_context(nc.semaphore(f"s_ow{k}")) for k in range(2)]
        s_s = es.enter_context(nc.semaphore("s_s"))      # DVE one-hot count
        s_smm = es.enter_context(nc.semaphore("s_smm"))  # PE pass-mm count
        s_vcp = es.enter_context(nc.semaphore("s_vcp"))  # DVE aggT copies
        s_omm = es.enter_context(nc.semaphore("s_omm"))  # PE final-mm count
        s_ocp = es.enter_context(nc.semaphore("s_ocp"))  # ACT relu count
        block = es.enter_context(nc.Block())

        (LD_COL00, LD_COL0, LD_COL1, LD_IO, LD_RL0, LD_RL1, LD_W, LD_W1,
         LD_BB, LD_ID) = range(10)

        @block.sync
        def _(sync):
            # Ramp-critical loads first: pair-0 idx slice gates the first
            # gather; iota + pair-0 rloc gate the first one-hot builds.
            sync.dma_start(iota_sb[:, :], io_d[:, :]).then_inc(s_ld[LD_IO], 16)
            sync.dma_start(
                rl_sb[:, 0:sl0], rl_d[:, 0:sl0]
            ).then_inc(s_ld[LD_RL0], 16)
            sync.dma_start(
                col_sb[0:32, ch0 * 8:], col_d[:, ch0 * 8:]
            ).then_inc(s_ld[LD_COL1], 16)
            sync.dma_start(
                rl_sb[:, sl0:], rl_d[:, sl0:]
            ).then_inc(s_ld[LD_RL1], 16)
            sync.dma_start(w_sb[:, 0, :], w_d[0]).then_inc(s_ld[LD_W], 16)
            sync.dma_start(w_sb[:, 1, :], w_d[1]).then_inc(s_ld[LD_W1], 16)
            sync.dma_start(bb_sb[:, :], bb_d[:, :]).then_inc(s_ld[LD_BB], 16)
            sync.dma_start(id_sb[:, :], id_d[:, :]).then_inc(s_ld[LD_ID], 16)
            for b in range(NB):
                sync.wait_ge(s_ocp, b + 1)
                sync.dma_start(
                    o_d[b * 128:(b + 1) * 128, :], o_sb[:, b % 2, :]
                ).then_inc(s_ow[b % 2], 16)

        @block.gpsimd
        def _(gpsimd):
            # self-load the ramp-critical idx slice via mainline SWDGE: no
            # cross-engine hop, ~0.8us earlier first gather
            gpsimd.dma_start(
                col_sb[0:32, 0:ch0 * 8], col_d[:, 0:ch0 * 8]
            ).then_inc(s_ld[LD_COL0], 16)
            gpsimd.wait_ge(s_ld[LD_COL0], 16)
            g_cross = next(g for g in range(ngat + 1) if GRP * g + GRP > ch0)
            for g in range(ngat):
                if g == g_cross:
                    gpsimd.wait_ge(s_ld[LD_COL1], 16)
                j0, j1 = GRP * g, min(GRP * g + GRP, nch)
                if j1 - VC > 0:
                    # val ring slots [j0%VC, ...) held chunks [j0-VC, j1-VC)
                    gpsimd.wait_ge(s_smm, consume_tgt(j1 - VC - 1))
                r = j0 % VC
                gpsimd.dma_gather(
                    val_sb[:, r:r + (j1 - j0), :],
                    x_d[:, :],
                    col_sb[:, j0 * 8:j1 * 8],
                    (j1 - j0) * 128,
                    (j1 - j0) * 128,
                    FIN,
                ).then_inc(s_gat[g % NSEM], 16)

        def pe_final(tensor, q):
            if q == 0:
                for k in (LD_W, LD_W1, LD_BB, LD_ID):
                    tensor.wait_ge(s_ld[k], 16)
            tensor.wait_ge(s_vcp, 2 * (q + 1))
            if q >= 2:
                tensor.wait_ge(s_ocp, q - 1)
            tensor.matmul(
                po[q % 2][:, 0:128], id_sb[:, :], bb_sb[:, :],
                start=True, stop=False,
            )
            tensor.matmul(
                po[q % 2][:, 0:128], at_sb[:, q % 2, 0, :], w_sb[:, 0, :],
                start=False, stop=False,
            )
            tensor.matmul(
                po[q % 2][:, 0:128], at_sb[:, q % 2, 1, :], w_sb[:, 1, :],
                start=False, stop=True,
            ).then_inc(s_omm, 1)

        @block.tensor
        def _(tensor):
            kk = 0
            waited_g = 0
            for q in range(NB):
                rng = q_range(q)
                for i, (j, np_, rlb) in enumerate(rng):
                    g = j // GRP
                    while waited_g <= g:
                        tensor.wait_ge(
                            s_gat[waited_g % NSEM],
                            16 * (waited_g // NSEM + 1),
                        )
                        waited_g += 1
                    if i == 0 and q >= 2:
                        # pa[q%2] fully copied out (block-slot q-2)
                        tensor.wait_ge(s_vcp, 2 * (q - 1))
                    for p in range(np_):
                        tensor.wait_ge(s_s, kk + 1)
                        st = i == 0 and p == 0
                        sp = i == len(rng) - 1 and p == np_ - 1
                        tensor.matmul(
                            pa[q % 2][0][:, 0:128],
                            val_sb[:, j % VC, 0:128],
                            s_sb[:, kk % S_BUFS, :],
                            start=st,
                            stop=sp,
                        )
                        tensor.matmul(
                            pa[q % 2][1][:, 0:128],
                            val_sb[:, j % VC, 128:256],
                            s_sb[:, kk % S_BUFS, :],
                            start=st,
                            stop=sp,
                        ).then_inc(s_smm, 1)
                        kk += 1
                if q >= 1:
                    pe_final(tensor, q - 1)
            pe_final(tensor, NB - 1)

        def dve_copies(vector, q):
            vector.wait_ge(s_smm, smm_after(q))
            if q >= 2:
                # at_sb[q%2] consumed by pe_final(q-2)
                vector.wait_ge(s_omm, q - 1)
            vector.tensor_copy(
                at_sb[:, q % 2, 0, :], pa[q % 2][0][:, 0:128]
            ).then_inc(s_vcp, 1)
            vector.tensor_copy(
                at_sb[:, q % 2, 1, :], pa[q % 2][1][:, 0:128]
            ).then_inc(s_vcp, 1)

        @block.vector
        def _(vector):
            vector.wait_ge(s_ld[LD_IO], 16)
            vector.wait_ge(s_ld[LD_RL0], 16)
            kk = 0
            for q in range(NB):
                if q == 2:
                    vector.wait_ge(s_ld[LD_RL1], 16)
                for i, (j, np_, rlb) in enumerate(q_range(q)):
                    for p in range(np_):
                        if kk >= S_BUFS:
                            vector.wait_ge(s_smm, kk - S_BUFS + 1)
                        slot = rlb + p
                        vector.tensor_scalar(
                            s_sb[:, kk % S_BUFS, :],
                            iota_sb[:, :],
                            rl_sb[:, slot:slot + 1],
                            None,
                            mybir.AluOpType.is_equal,
                        ).then_inc(s_s, 1)
                        kk += 1
                if q >= 1:
                    dve_copies(vector, q - 1)
            dve_copies(vector, NB - 1)

        @block.scalar
        def _(scalar):
            for q in range(NB):
                scalar.wait_ge(s_omm, q + 1)
                if q >= 2:
                    scalar.wait_ge(s_ow[q % 2], 16 * (q // 2))
                scalar.activation(
                    o_sb[:, q % 2, :],
                    po[q % 2][:, 0:128],
                    mybir.ActivationFunctionType.Relu,
                ).then_inc(s_ocp, 1)

    nc.compile()
    return nc


def _run(x, edge_index, weight, bias, trace=False):
    x16, w_t, bias_bc, ident, iota16, col16, rloc16, meta, pos = _host_prep(
        x, edge_index, weight, bias
    )
    nc = _build_program(meta)
    in_maps = [
        {
            "x16": x16,
            "w": w_t,
            "bb": bias_bc,
            "ident": ident,
            "iota": iota16,
            "col": np.ascontiguousarray(col16[c]),
            "rl": np.ascontiguousarray(rloc16[c]),
        }
        for c in range(NCORES)
    ]
    res = run_bass_kernel_spmd(nc, in_maps, list(range(NCORES)), trace=trace)
    out = np.concatenate([res.results[c]["out"] for c in range(NCORES)], axis=0)
    return np.ascontiguousarray(out[pos].astype(np.float32)), res


def kernel(x, edge_index, weight, bias):
    out, _ = _run(x, edge_index, weight, bias, trace=False)
    return out
